# revision 43
# baseline (speedup 1.0000x reference)
"""Trainium2 Bass kernel for nn_Attention_57423712748130.

Computation (per batch b):
  X4 = x[b] viewed (C=256, N=4096)   [raw reshape]
  Q4 = silu(BN(q_w @ X4))            (256, 4096)
  KV4 = silu(BN(kv_w @ Y4))          (128, 4096)
  q[n,h,d]  = Q4[n1, n0*256+h*64+d]      n = n1*16+n0
  k[m,h,d]  = KV4[m1, m0*512 + h*64+d]   m = m1*8+m0
  v[m,h,d]  = KV4[m1, m0*512+256+h*64+d]
  att = softmax(q k^T / 8); o = att v
  out rows [h*1024,(h+1)*1024) = O_h @ proj_w.T + proj_b
    where O_h[n2, n3*64+d] = o[4*n2+n3, d]

Sharding: 8 cores = (batch b in 0..3) x (head-pair hp in 0..1); each core
computes heads {2hp, 2hp+1} of batch b = rows [hp*2048,(hp+1)*2048) of out[b].

On-core strategy (v2 — ACT-exp is the per-core floor at ~62us):
 - every matmul operand is bf16 (host-cast inputs): FWL weight loads overlap,
   no f32r LDWEIGHTS serialization, input DMA bytes halved
 - conv/proj biases are preloaded into PSUM by K=1 ones x bias_row matmuls;
   silu epilogue is tanh (ACT, reads PSUM, scale 0.5) + one DVE
   scalar_tensor_tensor (t+1)*z -> bf16 (computes 2*silu(z); the 2x folds
   into exp scale 1/32 and the 2.0-fill of vext's ones column)
 - scores for hl=0/1 use PE row groups 0-1/2-3 (64-row weights at base
   partitions 0/64) so the pairs can run concurrently on the array
 - software-pipelined emission: scores(t2+1) interleave with av(t2) so the
   scalar engine's exp stream never waits on PE head-of-line blocking
 - denominators ride the av ones-column (row 64); reciprocal_approx_fast
   reads that PSUM row directly into rc_row; normalization grids are built
   by K=1 ones x rc_row matmuls into PSUM (no DRAM bounce, no gpsimd DMA)
 - proj outputs DMA straight from PSUM (bias preloaded by PE)
 - PE + ACT warm up on dummy ops during the input-DMA window (HAM throttle
   needs ~3.4us of sustained busy to release 2.4GHz; ACT table load ~2.7us)
"""

import ml_dtypes
import numpy as np

B = 4
N_TOK = 4096
C = 256
BN_EPS = 1e-5

_CACHE = {}


def _build():
    import concourse.bacc as bacc
    import concourse.bass as bass
    import concourse.tile as tile
    from concourse import mybir

    f32 = mybir.dt.float32
    bf16 = mybir.dt.bfloat16
    AF = mybir.ActivationFunctionType
    ALU = mybir.AluOpType

    nc = bacc.Bacc("TRN2", target_bir_lowering=False, debug=False, num_devices=8)

    xq = nc.dram_tensor("xq", [256, 2048], bf16, kind="ExternalInput")
    yk = nc.dram_tensor("yk", [256, 1024], bf16, kind="ExternalInput")
    yv = nc.dram_tensor("yv", [256, 1024], bf16, kind="ExternalInput")
    wq = nc.dram_tensor("wq", [256, 256], bf16, kind="ExternalInput")
    bq = nc.dram_tensor("bq", [1, 512], bf16, kind="ExternalInput")
    wkv = nc.dram_tensor("wkv", [256, 128], bf16, kind="ExternalInput")
    bkv = nc.dram_tensor("bkv", [1, 512], bf16, kind="ExternalInput")
    bkvcr = nc.dram_tensor("bkvcr", [1, 128], bf16, kind="ExternalInput")
    wp = nc.dram_tensor("wp", [256, 256], bf16, kind="ExternalInput")
    bp = nc.dram_tensor("bp", [1, 512], bf16, kind="ExternalInput")
    onesr = nc.dram_tensor("onesr", [1, 512], bf16, kind="ExternalInput")
    twod = nc.dram_tensor("twod", [1, 2048], bf16, kind="ExternalInput")
    out = nc.dram_tensor("out", [2048, 256], f32, kind="ExternalOutput")

    with tile.TileContext(nc) as tc:
        with (
            tc.tile_pool(name="const", bufs=1) as cp,
            tc.tile_pool(name="actt", bufs=3) as actt,
            tc.tile_pool(name="attp", bufs=16) as attp,
            tc.tile_pool(name="pssc", bufs=2, space="PSUM") as pssc,
            tc.tile_pool(name="psav", bufs=2, space="PSUM") as psav,
            tc.tile_pool(name="psqc", bufs=1, space="PSUM") as psqc,
            tc.tile_pool(name="pspj", bufs=1, space="PSUM") as pspj,
        ):
            # ---- small constants first on the scalar queue so the PE/ACT
            # warmup isn't gated on the bulk-input queues ----
            def load_row(t_dram, shape, tag):
                t = cp.tile(shape, bf16, tag=tag, name=tag)
                nc.scalar.dma_start(t[:], t_dram.ap())
                return t

            ones_sb = load_row(onesr, [1, 512], "ones")
            bq_sb = load_row(bq, [1, 512], "bqr")
            bkv_sb = load_row(bkv, [1, 512], "bkvr")
            bkvc_sb = load_row(bkvcr, [1, 128], "bkvcr")
            bp_sb = load_row(bp, [1, 512], "bpr")

            def load_w(t_dram, shape, tag, rows):
                t = cp.tile(shape, bf16, tag=tag, name=tag)
                nc.sync.dma_start(t[:], t_dram.ap()[rows[0]:rows[1], :])
                return t

            wkv_sb = [load_w(wkv, [128, 128], f"wkv{i}", (i * 128, (i + 1) * 128))
                      for i in range(2)]
            wq_sb = [load_w(wq, [128, 256], f"wq{i}", (i * 128, (i + 1) * 128))
                     for i in range(2)]
            wp_sb = [load_w(wp, [128, 256], f"wp{i}", (i * 128, (i + 1) * 128))
                     for i in range(2)]

            # ---- bulk inputs: split loads, k-conv inputs first ----
            def load_split(t_dram, shape, tag, row0, ncol, piece, eng):
                t = cp.tile(shape, bf16, tag=tag, name=tag)
                for j0 in range(0, ncol, piece):
                    eng.dma_start(
                        t[:, j0:j0 + piece],
                        t_dram.ap()[row0:row0 + 128, j0:j0 + piece])
                return t

            yk_sb = [load_split(yk, [128, 1024], f"yk{i}", i * 128, 1024, 1024,
                                nc.gpsimd) for i in range(2)]
            yv_sb = [load_split(yv, [128, 1024], f"yv{i}", i * 128, 1024, 1024,
                                nc.gpsimd) for i in range(2)]
            xq_sb = [load_split(xq, [128, 2048], f"xq{i}", i * 128, 2048, 1024,
                                nc.scalar if i else nc.sync) for i in range(2)]

            vext = cp.tile([128, 8, 2, 128], bf16, tag="vext")
            nc.gpsimd.dma_start(vext[:], twod.ap().partition_broadcast(128))

            # ---- prime the ACT exp/tanh table set during the input-DMA
            # window (table load ~2.7us; tanh+exp share one set) ----
            wsb = actt.tile([1, 512], f32, tag="warm_sb", name="warm_sb")
            nc.scalar.activation(wsb[:], ones_sb[:], AF.Tanh, scale=0.5)
            nc.scalar.activation(wsb[:], ones_sb[:], AF.Exp, scale=0.03125)

            # ---- conv epilogue: psum holds w@x + bias (preloaded).
            # t = tanh(ps/2) on ACT; dst = (t+1)*ps = 2*silu(ps) on DVE. ----
            def silu_into(ps, dst_ap, ps_ap, tag):
                shape = list(ps.shape)
                t = actt.tile(shape, f32, tag="silu_t", name=f"t_{tag}")
                nc.scalar.activation(t[:], ps, AF.Tanh, scale=0.5)
                nc.vector.scalar_tensor_tensor(
                    out=dst_ap, in0=t[:] if ps_ap is None else ps_ap(t[:]),
                    scalar=1.0, in1=ps if ps_ap is None else ps_ap(ps),
                    op0=ALU.add, op1=ALU.mult)

            # ---- kv conv (k part): kT[pp, m0, m1], pp = hl*64+d ----
            kT = cp.tile([128, 8, 128], bf16, tag="kT")

            def k_conv(mt):
                psw = pssc.tile([128, 1024], f32, tag="sc", name=f"psk{mt}")
                ps = psw[:, 0:512]
                nc.tensor.matmul(ps, lhsT=ones_sb[0:1, 0:128], rhs=bkv_sb[:],
                                 start=True, stop=False, skip_group_check=True)
                for mi in range(4):
                    m0 = 4 * mt + mi
                    sl = slice(mi * 128, (mi + 1) * 128)
                    for c0 in range(2):
                        nc.tensor.matmul(
                            ps[:, sl],
                            lhsT=yk_sb[c0][:, m0 * 128:(m0 + 1) * 128],
                            rhs=wkv_sb[c0][:],
                            start=False, stop=(c0 == 1),
                            skip_group_check=True)
                silu_into(
                    ps, kT[:, 4 * mt:4 * mt + 4, :].rearrange("p a b -> p (a b)"),
                    None, f"k{mt}")

            # ---- kv conv (v part): vext[m1, m0, hl, 0:64] = 2v ----
            def v_conv(jv):
                psw = pspj.tile([128, 512], f32, tag="pj", name=f"psv{jv}")
                ps = psw[:, 0:512]
                nc.tensor.matmul(ps, lhsT=bkvc_sb[:], rhs=ones_sb[:],
                                 start=True, stop=False)
                for c0 in range(2):
                    nc.tensor.matmul(
                        ps, lhsT=wkv_sb[c0][:],
                        rhs=yv_sb[c0][:, jv * 512:(jv + 1) * 512],
                        start=False, stop=(c0 == 1))
                silu_into(
                    ps, vext[:, jv * 4:(jv + 1) * 4, :, 0:64],
                    lambda a: a.rearrange("p (a h d) -> p a h d", a=4, h=2),
                    f"v{jv}")

            # ---- q conv: qT[pp, n0, n1]; t2=0 runs up front (pssc ring),
            # the rest interleave into the attention loop (psqc ring) so the
            # exp stream starts ~15us earlier ----
            qT = cp.tile([128, 16, 256], bf16, tag="qT")

            def q_conv(t2, pool, tag):
                psw = pool.tile([128, 512] if pool is psqc else [128, 1024],
                                f32, tag=tag, name=f"psq{t2}")
                ps = psw[:, 0:512]
                nc.tensor.matmul(ps, lhsT=ones_sb[0:1, 0:128], rhs=bq_sb[:],
                                 start=True, stop=False, skip_group_check=True)
                for nn in range(2):
                    n0 = 2 * t2 + nn
                    sl = slice(nn * 256, (nn + 1) * 256)
                    for c0 in range(2):
                        nc.tensor.matmul(
                            ps[:, sl],
                            lhsT=xq_sb[c0][:, n0 * 128:(n0 + 1) * 128],
                            rhs=wq_sb[c0][:],
                            start=False, stop=(c0 == 1),
                            skip_group_check=True)
                silu_into(
                    ps, qT[:, 2 * t2:2 * t2 + 2, :].rearrange("p a b -> p (a b)"),
                    None, f"q{t2}")

            # ---- attention state ----
            outun = [
                [cp.tile([128, 1024], bf16, tag=f"outun{hl}_{i}",
                         name=f"outun{hl}_{i}") for i in range(2)]
                for hl in range(2)
            ]
            rc_row = [cp.tile([1, 4096], f32, tag=f"rcrow{hl}", name=f"rcrow{hl}")
                      for hl in range(2)]

            att_tiles = {}
            grid_t = {}

            def emit_scores(t2, js=(0, 1, 2, 3)):
                """scores + exp for one n0-pair; hl=0/1 on row groups 0-1/2-3."""
                for j in js:
                    scps = {}
                    for hl in range(2):
                        scps[hl] = pssc.tile([128, 1024], f32, tag="sc",
                                             name=f"scp{hl}_{t2}_{j}")
                    for mi in range(2):
                        m0 = 2 * j + mi
                        for hl in range(2):
                            r0, r1 = hl * 64, (hl + 1) * 64
                            nc.tensor.matmul(
                                scps[hl][:, mi * 512:(mi + 1) * 512],
                                lhsT=kT[r0:r1, m0, :],
                                rhs=qT[r0:r1, 2 * t2:2 * t2 + 2, :],
                                start=True, stop=True)
                    for hl in range(2):
                        a = attp.tile([128, 1024], bf16, tag="att",
                                      name=f"att{hl}_{t2}_{j}")
                        # scoresT = 4*q.k ; want exp(q.k/8) -> scale 1/32
                        nc.scalar.activation(
                            a[:], scps[hl][:], AF.Exp, scale=0.03125)
                        att_tiles[(t2, j, hl)] = a

            def emit_av_pair(t2, j):
                """av matmuls for m0 = 2j, 2j+1, both heads."""
                for hl in range(2):
                    if j == 0:
                        ops = psav.tile([128, 512], f32, tag="ops",
                                        name=f"ops{hl}_{t2}")
                        att_tiles[("ops", t2, hl)] = ops
                    ops = att_tiles[("ops", t2, hl)]
                    for mi in range(2):
                        m0 = 2 * j + mi
                        nc.tensor.matmul(
                            ops[:], lhsT=vext[:, m0, hl, :],
                            rhs=att_tiles[(t2, m0 // 2, hl)][
                                :, (m0 % 2) * 512:(m0 % 2 + 1) * 512],
                            start=(m0 == 0), stop=(m0 == 7))

            def emit_evac(t2):
                """o -> outun (cast), denominators -> rc via fast reciprocal."""
                c0, q0 = t2 & 1, t2 >> 1
                for hl in range(2):
                    ops = att_tiles[("ops", t2, hl)]
                    sl = slice(t2 * 512, (t2 + 1) * 512)
                    sums = actt.tile([1, 512], f32, tag="sums",
                                     name=f"sums{hl}_{t2}")
                    nc.vector.tensor_copy(sums[:], ops[64:65, :])
                    nc.vector.reciprocal_approx_fast(
                        out=rc_row[hl][0:1, sl], in_=sums[:])
                    g = actt.tile([128, 512], f32, tag=f"grid{hl}",
                                  name=f"gr{hl}_{t2}")
                    nc.gpsimd.partition_broadcast(g[:], rc_row[hl][0:1, sl])
                    grid_t[(t2, hl)] = g
                    for nn in range(2):
                        dst = outun[hl][c0][
                            nn * 64:nn * 64 + 64, q0 * 256:(q0 + 1) * 256]
                        nc.vector.tensor_copy(
                            dst, ops[0:64, nn * 256:(nn + 1) * 256])

            def emit_tail(q0):
                """normalize outun columns of quarter q0 and project."""
                for hl in range(2):
                    sl = slice(q0 * 256, (q0 + 1) * 256)
                    for c0 in range(2):
                        g = grid_t.pop((2 * q0 + c0, hl))
                        for band in range(2):
                            pr = slice(band * 64, (band + 1) * 64)
                            nc.vector.tensor_mul(
                                outun[hl][c0][pr, sl], outun[hl][c0][pr, sl],
                                g[pr, band * 256:(band + 1) * 256])
                    pjpool = psqc if (q0 == 3 and hl == 1) else pspj
                    ps2 = pjpool.tile([128, 512], f32,
                                      tag="qc" if (q0 == 3 and hl == 1) else "pj",
                                      name=f"psproj{hl}_{q0}")
                    nc.tensor.matmul(ps2[:], lhsT=ones_sb[0:1, 0:128],
                                     rhs=bp_sb[:], start=True, stop=False,
                                     skip_group_check=True)
                    for half in range(2):
                        fc = 2 * q0 + half
                        sl = slice(half * 256, (half + 1) * 256)
                        for c0 in range(2):
                            nc.tensor.matmul(
                                ps2[:, sl],
                                lhsT=outun[hl][c0][:, fc * 128:(fc + 1) * 128],
                                rhs=wp_sb[c0][:],
                                start=False, stop=(c0 == 1),
                                skip_group_check=True)
                    osb = actt.tile([128, 512], f32, tag="osb",
                                    name=f"osb{hl}_{q0}")
                    nc.vector.tensor_copy(osb[:], ps2[:])
                    dstap = bass.AP(
                        tensor=out,
                        offset=(hl * 1024 + q0) * 256,
                        ap=[[4 * 256, 128], [512 * 256, 2], [1, 256]])
                    nc.sync.dma_start(
                        dstap, osb[:].rearrange("p (h c) -> p h c", h=2))

            # ---- software-pipelined attention: av(t2) first (runnable
            # early in the window), then scores(t2+1) (ring-gated to late
            # window) so the in-order PE queue never head-of-line blocks ----
            k_conv(0)
            k_conv(1)
            v_conv(0)
            v_conv(1)
            q_conv(0, pssc, "sc")
            q_conv(1, psqc, "qc")
            emit_scores(0)
            for t2 in range(8):
                for j in range(3):
                    emit_av_pair(t2, j)
                if t2 < 6:
                    q_conv(t2 + 2, psqc, "qc")
                if t2 < 7:
                    emit_scores(t2 + 1)
                emit_av_pair(t2, 3)
                emit_evac(t2)
                if t2 & 1:
                    emit_tail(t2 >> 1)

    nc.compile()
    return nc


def _prep_inputs(x, y, q_w, q_gamma, q_beta, q_mean, q_var,
                 kv_w, kv_gamma, kv_beta, kv_mean, kv_var, proj_w, proj_b):
    f = np.float32
    bf = ml_dtypes.bfloat16
    x = np.ascontiguousarray(np.asarray(x, f))
    y = np.ascontiguousarray(np.asarray(y, f))

    gq = np.asarray(q_gamma, f) / np.sqrt(np.asarray(q_var, f) + BN_EPS)
    bq_full = np.asarray(q_beta, f) - np.asarray(q_mean, f) * gq
    wq_host = np.ascontiguousarray((np.asarray(q_w, f) * gq[:, None]).T).astype(bf)

    gkv = np.asarray(kv_gamma, f) / np.sqrt(np.asarray(kv_var, f) + BN_EPS)
    bkv_full = np.asarray(kv_beta, f) - np.asarray(kv_mean, f) * gkv
    wkv_host = np.ascontiguousarray(
        (np.asarray(kv_w, f) * gkv[:, None]).T).astype(bf)

    wp_host = np.ascontiguousarray(np.asarray(proj_w, f).T).astype(bf)
    bp_host = np.asarray(proj_b, f)

    bq2 = np.tile(bq_full[None, :], (1, 2)).astype(bf)
    bkv2 = np.tile(bkv_full[None, :], (1, 4)).astype(bf)
    bkvc2 = bkv_full[None, :].astype(bf)
    bp2 = np.tile(bp_host[None, :], (1, 2)).astype(bf)
    ones2 = np.ones((1, 512), bf)
    twod2 = np.zeros((1, 2048), np.float32)
    twod2.reshape(16, 128)[:, 64] = 2.0
    twod2 = twod2.astype(bf)

    in_maps = []
    for core in range(8):
        b, hp = core // 2, core % 2
        X4 = x[b].reshape(C, N_TOK)
        Y4 = y[b].reshape(C, N_TOK)
        xqa = np.ascontiguousarray(
            X4.reshape(C, 16, 256)[:, :, hp * 128:(hp + 1) * 128]
        ).reshape(C, 2048).astype(bf)
        Y8 = Y4.reshape(C, 8, 512)
        yka = np.ascontiguousarray(
            Y8[:, :, hp * 128:(hp + 1) * 128]).reshape(C, 1024).astype(bf)
        yva = np.ascontiguousarray(
            Y8[:, :, 256 + hp * 128:256 + (hp + 1) * 128]
        ).reshape(C, 1024).astype(bf)
        in_maps.append({
            "xq": xqa, "yk": yka, "yv": yva,
            "wq": wq_host, "bq": bq2,
            "wkv": wkv_host, "bkv": bkv2, "bkvcr": bkvc2,
            "wp": wp_host, "bp": bp2,
            "onesr": ones2, "twod": twod2,
        })
    return in_maps


def _get_nc():
    if "nc" not in _CACHE:
        _CACHE["nc"] = _build()
    return _CACHE["nc"]


def kernel(x, y, H=64, W=64, q_w=None, q_gamma=None, q_beta=None, q_mean=None,
           q_var=None, kv_w=None, kv_gamma=None, kv_beta=None, kv_mean=None,
           kv_var=None, proj_w=None, proj_b=None, _trace=False):
    from concourse.bass_utils import run_bass_kernel_spmd

    nc = _get_nc()
    in_maps = _prep_inputs(x, y, q_w, q_gamma, q_beta, q_mean, q_var,
                           kv_w, kv_gamma, kv_beta, kv_mean, kv_var,
                           proj_w, proj_b)
    kw = {}
    if _trace:
        kw = {"trace": True, "trace_cores": list(range(8))}
    res = run_bass_kernel_spmd(nc, in_maps, list(range(8)), **kw)
    outa = np.empty((B, N_TOK, C), np.float32)
    for core in range(8):
        b, hp = core // 2, core % 2
        outa[b, hp * 2048:(hp + 1) * 2048, :] = res.results[core]["out"]
    if _trace:
        return outa, res
    return outa


# revision 44
# speedup vs baseline: 1.0431x; 1.0431x over previous
"""Trainium2 Bass kernel for nn_Attention_57423712748130.

Computation (per batch b):
  X4 = x[b] viewed (C=256, N=4096)   [raw reshape]
  Q4 = silu(BN(q_w @ X4))            (256, 4096)
  KV4 = silu(BN(kv_w @ Y4))          (128, 4096)
  q[n,h,d]  = Q4[n1, n0*256+h*64+d]      n = n1*16+n0
  k[m,h,d]  = KV4[m1, m0*512 + h*64+d]   m = m1*8+m0
  v[m,h,d]  = KV4[m1, m0*512+256+h*64+d]
  att = softmax(q k^T / 8); o = att v
  out rows [h*1024,(h+1)*1024) = O_h @ proj_w.T + proj_b
    where O_h[n2, n3*64+d] = o[4*n2+n3, d]

Sharding: 8 cores = (batch b in 0..3) x (head-pair hp in 0..1); each core
computes heads {2hp, 2hp+1} of batch b = rows [hp*2048,(hp+1)*2048) of out[b].

On-core strategy (v2 — ACT-exp is the per-core floor at ~62us):
 - every matmul operand is bf16 (host-cast inputs): FWL weight loads overlap,
   no f32r LDWEIGHTS serialization, input DMA bytes halved
 - conv/proj biases are preloaded into PSUM by K=1 ones x bias_row matmuls;
   silu epilogue is tanh (ACT, reads PSUM, scale 0.5) + one DVE
   scalar_tensor_tensor (t+1)*z -> bf16 (computes 2*silu(z); the 2x folds
   into exp scale 1/32 and the 2.0-fill of vext's ones column)
 - scores for hl=0/1 use PE row groups 0-1/2-3 (64-row weights at base
   partitions 0/64) so the pairs can run concurrently on the array
 - software-pipelined emission: scores(t2+1) interleave with av(t2) so the
   scalar engine's exp stream never waits on PE head-of-line blocking
 - denominators ride the av ones-column (row 64); reciprocal_approx_fast
   reads that PSUM row directly into rc_row; normalization grids are built
   by K=1 ones x rc_row matmuls into PSUM (no DRAM bounce, no gpsimd DMA)
 - proj outputs DMA straight from PSUM (bias preloaded by PE)
 - PE + ACT warm up on dummy ops during the input-DMA window (HAM throttle
   needs ~3.4us of sustained busy to release 2.4GHz; ACT table load ~2.7us)
"""

import ml_dtypes
import numpy as np

B = 4
N_TOK = 4096
C = 256
BN_EPS = 1e-5

_CACHE = {}


def _build():
    import concourse.bacc as bacc
    import concourse.bass as bass
    import concourse.tile as tile
    from concourse import mybir

    f32 = mybir.dt.float32
    bf16 = mybir.dt.bfloat16
    AF = mybir.ActivationFunctionType
    ALU = mybir.AluOpType

    nc = bacc.Bacc("TRN2", target_bir_lowering=False, debug=False, num_devices=8)

    xq = nc.dram_tensor("xq", [256, 2048], bf16, kind="ExternalInput")
    yk = nc.dram_tensor("yk", [256, 1024], bf16, kind="ExternalInput")
    yv = nc.dram_tensor("yv", [256, 1024], bf16, kind="ExternalInput")
    wq = nc.dram_tensor("wq", [256, 256], bf16, kind="ExternalInput")
    bq = nc.dram_tensor("bq", [1, 512], bf16, kind="ExternalInput")
    wkv = nc.dram_tensor("wkv", [256, 128], bf16, kind="ExternalInput")
    bkv = nc.dram_tensor("bkv", [1, 512], bf16, kind="ExternalInput")
    bkvcr = nc.dram_tensor("bkvcr", [1, 128], bf16, kind="ExternalInput")
    wp = nc.dram_tensor("wp", [256, 256], bf16, kind="ExternalInput")
    bp = nc.dram_tensor("bp", [1, 512], bf16, kind="ExternalInput")
    onesr = nc.dram_tensor("onesr", [1, 512], bf16, kind="ExternalInput")
    twod = nc.dram_tensor("twod", [1, 2048], bf16, kind="ExternalInput")
    out = nc.dram_tensor("out", [2048, 256], f32, kind="ExternalOutput")

    with tile.TileContext(nc) as tc:
        with (
            tc.tile_pool(name="const", bufs=1) as cp,
            tc.tile_pool(name="actt", bufs=3) as actt,
            tc.tile_pool(name="attp", bufs=16) as attp,
            tc.tile_pool(name="pssc", bufs=2, space="PSUM") as pssc,
            tc.tile_pool(name="psav", bufs=2, space="PSUM") as psav,
            tc.tile_pool(name="psqc", bufs=1, space="PSUM") as psqc,
            tc.tile_pool(name="pspj", bufs=1, space="PSUM") as pspj,
        ):
            # ---- small constants first on the scalar queue so the PE/ACT
            # warmup isn't gated on the bulk-input queues ----
            def load_row(t_dram, shape, tag):
                t = cp.tile(shape, bf16, tag=tag, name=tag)
                nc.scalar.dma_start(t[:], t_dram.ap())
                return t

            ones_sb = load_row(onesr, [1, 512], "ones")
            bq_sb = load_row(bq, [1, 512], "bqr")
            bkv_sb = load_row(bkv, [1, 512], "bkvr")
            bkvc_sb = load_row(bkvcr, [1, 128], "bkvcr")
            bp_sb = load_row(bp, [1, 512], "bpr")

            def load_w(t_dram, shape, tag, rows):
                t = cp.tile(shape, bf16, tag=tag, name=tag)
                nc.sync.dma_start(t[:], t_dram.ap()[rows[0]:rows[1], :])
                return t

            wkv_sb = [load_w(wkv, [128, 128], f"wkv{i}", (i * 128, (i + 1) * 128))
                      for i in range(2)]
            wq_sb = [load_w(wq, [128, 256], f"wq{i}", (i * 128, (i + 1) * 128))
                     for i in range(2)]
            wp_sb = [load_w(wp, [128, 256], f"wp{i}", (i * 128, (i + 1) * 128))
                     for i in range(2)]

            # ---- bulk inputs: split loads, k-conv inputs first ----
            def load_split(t_dram, shape, tag, row0, ncol, piece, eng):
                t = cp.tile(shape, bf16, tag=tag, name=tag)
                for j0 in range(0, ncol, piece):
                    eng.dma_start(
                        t[:, j0:j0 + piece],
                        t_dram.ap()[row0:row0 + 128, j0:j0 + piece])
                return t

            yk_sb = [load_split(yk, [128, 1024], f"yk{i}", i * 128, 1024, 1024,
                                nc.gpsimd) for i in range(2)]
            yv_sb = [load_split(yv, [128, 1024], f"yv{i}", i * 128, 1024, 1024,
                                nc.gpsimd) for i in range(2)]
            xq_sb = [load_split(xq, [128, 2048], f"xq{i}", i * 128, 2048, 1024,
                                nc.scalar if i else nc.sync) for i in range(2)]

            vext = cp.tile([128, 8, 2, 128], bf16, tag="vext")
            nc.gpsimd.dma_start(vext[:], twod.ap().partition_broadcast(128))

            # ---- prime the ACT exp/tanh table set during the input-DMA
            # window (table load ~2.7us; tanh+exp share one set) ----
            wsb = actt.tile([1, 512], f32, tag="warm_sb", name="warm_sb")
            nc.scalar.activation(wsb[:], ones_sb[:], AF.Tanh, scale=0.5)
            nc.scalar.activation(wsb[:], ones_sb[:], AF.Exp, scale=0.03125)

            # ---- conv epilogue: psum holds w@x + bias (preloaded).
            # t = tanh(ps/2) on ACT; dst = (t+1)*ps = 2*silu(ps) on DVE. ----
            def silu_into(ps, dst_ap, ps_ap, tag):
                shape = list(ps.shape)
                t = actt.tile(shape, f32, tag="silu_t", name=f"t_{tag}")
                nc.scalar.activation(t[:], ps, AF.Tanh, scale=0.5)
                nc.vector.scalar_tensor_tensor(
                    out=dst_ap, in0=t[:] if ps_ap is None else ps_ap(t[:]),
                    scalar=1.0, in1=ps if ps_ap is None else ps_ap(ps),
                    op0=ALU.add, op1=ALU.mult)

            # ---- kv conv (k part): kT[pp, m0, m1], pp = hl*64+d ----
            kT = cp.tile([128, 8, 128], bf16, tag="kT")

            def k_conv(mt):
                psw = pssc.tile([128, 1024], f32, tag="sc", name=f"psk{mt}")
                ps = psw[:, 0:512]
                nc.tensor.matmul(ps, lhsT=ones_sb[0:1, 0:128], rhs=bkv_sb[:],
                                 start=True, stop=False, skip_group_check=True)
                for mi in range(4):
                    m0 = 4 * mt + mi
                    sl = slice(mi * 128, (mi + 1) * 128)
                    for c0 in range(2):
                        nc.tensor.matmul(
                            ps[:, sl],
                            lhsT=yk_sb[c0][:, m0 * 128:(m0 + 1) * 128],
                            rhs=wkv_sb[c0][:],
                            start=False, stop=(c0 == 1),
                            skip_group_check=True)
                silu_into(
                    ps, kT[:, 4 * mt:4 * mt + 4, :].rearrange("p a b -> p (a b)"),
                    None, f"k{mt}")

            # ---- kv conv (v part): vext[m1, m0, hl, 0:64] = 2v ----
            def v_conv(jv):
                psw = pspj.tile([128, 512], f32, tag="pj", name=f"psv{jv}")
                ps = psw[:, 0:512]
                nc.tensor.matmul(ps, lhsT=bkvc_sb[:], rhs=ones_sb[:],
                                 start=True, stop=False)
                for c0 in range(2):
                    nc.tensor.matmul(
                        ps, lhsT=wkv_sb[c0][:],
                        rhs=yv_sb[c0][:, jv * 512:(jv + 1) * 512],
                        start=False, stop=(c0 == 1))
                silu_into(
                    ps, vext[:, jv * 4:(jv + 1) * 4, :, 0:64],
                    lambda a: a.rearrange("p (a h d) -> p a h d", a=4, h=2),
                    f"v{jv}")

            # ---- q conv: qT[pp, n0, n1]; t2=0 runs up front (pssc ring),
            # the rest interleave into the attention loop (psqc ring) so the
            # exp stream starts ~15us earlier ----
            qT = cp.tile([128, 16, 256], bf16, tag="qT")

            def q_conv(t2, pool, tag):
                psw = pool.tile([128, 512] if pool is psqc else [128, 1024],
                                f32, tag=tag, name=f"psq{t2}")
                ps = psw[:, 0:512]
                nc.tensor.matmul(ps, lhsT=ones_sb[0:1, 0:128], rhs=bq_sb[:],
                                 start=True, stop=False, skip_group_check=True)
                for nn in range(2):
                    n0 = 2 * t2 + nn
                    sl = slice(nn * 256, (nn + 1) * 256)
                    for c0 in range(2):
                        nc.tensor.matmul(
                            ps[:, sl],
                            lhsT=xq_sb[c0][:, n0 * 128:(n0 + 1) * 128],
                            rhs=wq_sb[c0][:],
                            start=False, stop=(c0 == 1),
                            skip_group_check=True)
                silu_into(
                    ps, qT[:, 2 * t2:2 * t2 + 2, :].rearrange("p a b -> p (a b)"),
                    None, f"q{t2}")

            # ---- attention state ----
            outun = [
                [cp.tile([128, 1024], bf16, tag=f"outun{hl}_{i}",
                         name=f"outun{hl}_{i}") for i in range(2)]
                for hl in range(2)
            ]
            rc_row = [cp.tile([1, 4096], f32, tag=f"rcrow{hl}", name=f"rcrow{hl}")
                      for hl in range(2)]

            att_tiles = {}
            grid_t = {}

            def emit_scores(t2, js=(0, 1, 2, 3)):
                """scores + exp for one n0-pair; hl=0/1 on row groups 0-1/2-3."""
                for j in js:
                    scps = {}
                    for hl in range(2):
                        scps[hl] = pssc.tile([128, 1024], f32, tag="sc",
                                             name=f"scp{hl}_{t2}_{j}")
                    for mi in range(2):
                        m0 = 2 * j + mi
                        for hl in range(2):
                            r0, r1 = hl * 64, (hl + 1) * 64
                            nc.tensor.matmul(
                                scps[hl][:, mi * 512:(mi + 1) * 512],
                                lhsT=kT[r0:r1, m0, :],
                                rhs=qT[r0:r1, 2 * t2:2 * t2 + 2, :],
                                start=True, stop=True)
                    for hl in range(2):
                        a = attp.tile([128, 1024], bf16, tag="att",
                                      name=f"att{hl}_{t2}_{j}")
                        # scoresT = 4*q.k ; want exp(q.k/8) -> scale 1/32
                        nc.scalar.activation(
                            a[:], scps[hl][:], AF.Exp, scale=0.03125)
                        att_tiles[(t2, j, hl)] = a

            def emit_av_pair(t2, j):
                """av matmuls for m0 = 2j, 2j+1, both heads."""
                for hl in range(2):
                    if j == 0:
                        ops = psav.tile([128, 512], f32, tag="ops",
                                        name=f"ops{hl}_{t2}")
                        att_tiles[("ops", t2, hl)] = ops
                    ops = att_tiles[("ops", t2, hl)]
                    for mi in range(2):
                        m0 = 2 * j + mi
                        nc.tensor.matmul(
                            ops[:], lhsT=vext[:, m0, hl, :],
                            rhs=att_tiles[(t2, m0 // 2, hl)][
                                :, (m0 % 2) * 512:(m0 % 2 + 1) * 512],
                            start=(m0 == 0), stop=(m0 == 7))

            def emit_evac(t2):
                """o -> outun (cast), denominators -> rc via fast reciprocal."""
                c0, q0 = t2 & 1, t2 >> 1
                for hl in range(2):
                    ops = att_tiles[("ops", t2, hl)]
                    sl = slice(t2 * 512, (t2 + 1) * 512)
                    sums = actt.tile([1, 512], f32, tag="sums",
                                     name=f"sums{hl}_{t2}")
                    nc.vector.tensor_copy(sums[:], ops[64:65, :])
                    nc.vector.reciprocal_approx_fast(
                        out=rc_row[hl][0:1, sl], in_=sums[:])
                    g = actt.tile([128, 512], f32, tag=f"grid{hl}",
                                  name=f"gr{hl}_{t2}")
                    nc.gpsimd.partition_broadcast(g[:], rc_row[hl][0:1, sl])
                    grid_t[(t2, hl)] = g
                    for nn in range(2):
                        dst = outun[hl][c0][
                            nn * 64:nn * 64 + 64, q0 * 256:(q0 + 1) * 256]
                        nc.vector.tensor_copy(
                            dst, ops[0:64, nn * 256:(nn + 1) * 256])

            def emit_tail(q0):
                """normalize outun columns of quarter q0 and project."""
                for hl in range(2):
                    sl = slice(q0 * 256, (q0 + 1) * 256)
                    for c0 in range(2):
                        g = grid_t.pop((2 * q0 + c0, hl))
                        for band in range(2):
                            pr = slice(band * 64, (band + 1) * 64)
                            nc.vector.tensor_mul(
                                outun[hl][c0][pr, sl], outun[hl][c0][pr, sl],
                                g[pr, band * 256:(band + 1) * 256])
                    pjpool = psqc if (q0 == 3 and hl == 1) else pspj
                    ps2 = pjpool.tile([128, 512], f32,
                                      tag="qc" if (q0 == 3 and hl == 1) else "pj",
                                      name=f"psproj{hl}_{q0}")
                    nc.tensor.matmul(ps2[:], lhsT=ones_sb[0:1, 0:128],
                                     rhs=bp_sb[:], start=True, stop=False,
                                     skip_group_check=True)
                    for half in range(2):
                        fc = 2 * q0 + half
                        sl = slice(half * 256, (half + 1) * 256)
                        for c0 in range(2):
                            nc.tensor.matmul(
                                ps2[:, sl],
                                lhsT=outun[hl][c0][:, fc * 128:(fc + 1) * 128],
                                rhs=wp_sb[c0][:],
                                start=False, stop=(c0 == 1),
                                skip_group_check=True)
                    osb = actt.tile([128, 512], f32, tag="osb",
                                    name=f"osb{hl}_{q0}")
                    nc.vector.tensor_copy(osb[:], ps2[:])
                    dstap = bass.AP(
                        tensor=out,
                        offset=(hl * 1024 + q0) * 256,
                        ap=[[4 * 256, 128], [512 * 256, 2], [1, 256]])
                    nc.sync.dma_start(
                        dstap, osb[:].rearrange("p (h c) -> p h c", h=2))

            # ---- software-pipelined attention: av(t2) first (runnable
            # early in the window), then scores(t2+1) (ring-gated to late
            # window) so the in-order PE queue never head-of-line blocks ----
            k_conv(0)
            k_conv(1)
            v_conv(0)
            v_conv(1)
            q_conv(0, pssc, "sc")
            q_conv(1, psqc, "qc")
            emit_scores(0)
            for t2 in range(8):
                for j in range(4):
                    emit_av_pair(t2, j)
                if t2 < 6:
                    q_conv(t2 + 2, psqc, "qc")
                if t2 < 7:
                    emit_scores(t2 + 1)
                emit_evac(t2)
                if t2 & 1:
                    emit_tail(t2 >> 1)

    nc.compile()
    return nc


def _prep_inputs(x, y, q_w, q_gamma, q_beta, q_mean, q_var,
                 kv_w, kv_gamma, kv_beta, kv_mean, kv_var, proj_w, proj_b):
    f = np.float32
    bf = ml_dtypes.bfloat16
    x = np.ascontiguousarray(np.asarray(x, f))
    y = np.ascontiguousarray(np.asarray(y, f))

    gq = np.asarray(q_gamma, f) / np.sqrt(np.asarray(q_var, f) + BN_EPS)
    bq_full = np.asarray(q_beta, f) - np.asarray(q_mean, f) * gq
    wq_host = np.ascontiguousarray((np.asarray(q_w, f) * gq[:, None]).T).astype(bf)

    gkv = np.asarray(kv_gamma, f) / np.sqrt(np.asarray(kv_var, f) + BN_EPS)
    bkv_full = np.asarray(kv_beta, f) - np.asarray(kv_mean, f) * gkv
    wkv_host = np.ascontiguousarray(
        (np.asarray(kv_w, f) * gkv[:, None]).T).astype(bf)

    wp_host = np.ascontiguousarray(np.asarray(proj_w, f).T).astype(bf)
    bp_host = np.asarray(proj_b, f)

    bq2 = np.tile(bq_full[None, :], (1, 2)).astype(bf)
    bkv2 = np.tile(bkv_full[None, :], (1, 4)).astype(bf)
    bkvc2 = bkv_full[None, :].astype(bf)
    bp2 = np.tile(bp_host[None, :], (1, 2)).astype(bf)
    ones2 = np.ones((1, 512), bf)
    twod2 = np.zeros((1, 2048), np.float32)
    twod2.reshape(16, 128)[:, 64] = 2.0
    twod2 = twod2.astype(bf)

    in_maps = []
    for core in range(8):
        b, hp = core // 2, core % 2
        X4 = x[b].reshape(C, N_TOK)
        Y4 = y[b].reshape(C, N_TOK)
        xqa = np.ascontiguousarray(
            X4.reshape(C, 16, 256)[:, :, hp * 128:(hp + 1) * 128]
        ).reshape(C, 2048).astype(bf)
        Y8 = Y4.reshape(C, 8, 512)
        yka = np.ascontiguousarray(
            Y8[:, :, hp * 128:(hp + 1) * 128]).reshape(C, 1024).astype(bf)
        yva = np.ascontiguousarray(
            Y8[:, :, 256 + hp * 128:256 + (hp + 1) * 128]
        ).reshape(C, 1024).astype(bf)
        in_maps.append({
            "xq": xqa, "yk": yka, "yv": yva,
            "wq": wq_host, "bq": bq2,
            "wkv": wkv_host, "bkv": bkv2, "bkvcr": bkvc2,
            "wp": wp_host, "bp": bp2,
            "onesr": ones2, "twod": twod2,
        })
    return in_maps


def _get_nc():
    if "nc" not in _CACHE:
        _CACHE["nc"] = _build()
    return _CACHE["nc"]


def kernel(x, y, H=64, W=64, q_w=None, q_gamma=None, q_beta=None, q_mean=None,
           q_var=None, kv_w=None, kv_gamma=None, kv_beta=None, kv_mean=None,
           kv_var=None, proj_w=None, proj_b=None, _trace=False):
    from concourse.bass_utils import run_bass_kernel_spmd

    nc = _get_nc()
    in_maps = _prep_inputs(x, y, q_w, q_gamma, q_beta, q_mean, q_var,
                           kv_w, kv_gamma, kv_beta, kv_mean, kv_var,
                           proj_w, proj_b)
    kw = {}
    if _trace:
        kw = {"trace": True, "trace_cores": list(range(8))}
    res = run_bass_kernel_spmd(nc, in_maps, list(range(8)), **kw)
    outa = np.empty((B, N_TOK, C), np.float32)
    for core in range(8):
        b, hp = core // 2, core % 2
        outa[b, hp * 2048:(hp + 1) * 2048, :] = res.results[core]["out"]
    if _trace:
        return outa, res
    return outa


# revision 47
# speedup vs baseline: 1.0911x; 1.0461x over previous
"""Trainium2 Bass kernel for nn_Attention_57423712748130.

Computation (per batch b):
  X4 = x[b] viewed (C=256, N=4096)   [raw reshape]
  Q4 = silu(BN(q_w @ X4))            (256, 4096)
  KV4 = silu(BN(kv_w @ Y4))          (128, 4096)
  q[n,h,d]  = Q4[n1, n0*256+h*64+d]      n = n1*16+n0
  k[m,h,d]  = KV4[m1, m0*512 + h*64+d]   m = m1*8+m0
  v[m,h,d]  = KV4[m1, m0*512+256+h*64+d]
  att = softmax(q k^T / 8); o = att v
  out rows [h*1024,(h+1)*1024) = O_h @ proj_w.T + proj_b
    where O_h[n2, n3*64+d] = o[4*n2+n3, d]

Sharding: 8 cores = (batch b in 0..3) x (head-pair hp in 0..1); each core
computes heads {2hp, 2hp+1} of batch b = rows [hp*2048,(hp+1)*2048) of out[b].

On-core strategy (v2 — ACT-exp is the per-core floor at ~62us):
 - every matmul operand is bf16 (host-cast inputs): FWL weight loads overlap,
   no f32r LDWEIGHTS serialization, input DMA bytes halved
 - conv/proj biases are preloaded into PSUM by K=1 ones x bias_row matmuls;
   silu epilogue is tanh (ACT, reads PSUM, scale 0.5) + one DVE
   scalar_tensor_tensor (t+1)*z -> bf16 (computes 2*silu(z); the 2x folds
   into exp scale 1/32 and the 2.0-fill of vext's ones column)
 - scores for hl=0/1 use PE row groups 0-1/2-3 (64-row weights at base
   partitions 0/64) so the pairs can run concurrently on the array
 - software-pipelined emission: scores(t2+1) interleave with av(t2) so the
   scalar engine's exp stream never waits on PE head-of-line blocking
 - denominators ride the av ones-column (row 64); reciprocal_approx_fast
   reads that PSUM row directly into rc_row; normalization grids are built
   by K=1 ones x rc_row matmuls into PSUM (no DRAM bounce, no gpsimd DMA)
 - proj outputs DMA straight from PSUM (bias preloaded by PE)
 - PE + ACT warm up on dummy ops during the input-DMA window (HAM throttle
   needs ~3.4us of sustained busy to release 2.4GHz; ACT table load ~2.7us)
"""

import ml_dtypes
import numpy as np

B = 4
N_TOK = 4096
C = 256
BN_EPS = 1e-5

_CACHE = {}


def _build():
    import concourse.bacc as bacc
    import concourse.bass as bass
    import concourse.tile as tile
    from concourse import mybir

    f32 = mybir.dt.float32
    bf16 = mybir.dt.bfloat16
    AF = mybir.ActivationFunctionType
    ALU = mybir.AluOpType

    nc = bacc.Bacc("TRN2", target_bir_lowering=False, debug=False, num_devices=8)

    xq = nc.dram_tensor("xq", [256, 2048], bf16, kind="ExternalInput")
    yk = nc.dram_tensor("yk", [256, 1024], bf16, kind="ExternalInput")
    yv = nc.dram_tensor("yv", [256, 1024], bf16, kind="ExternalInput")
    wq = nc.dram_tensor("wq", [256, 256], bf16, kind="ExternalInput")
    bq = nc.dram_tensor("bq", [1, 512], bf16, kind="ExternalInput")
    wkv = nc.dram_tensor("wkv", [256, 128], bf16, kind="ExternalInput")
    bkv = nc.dram_tensor("bkv", [1, 512], bf16, kind="ExternalInput")
    bkvcr = nc.dram_tensor("bkvcr", [1, 128], bf16, kind="ExternalInput")
    wp = nc.dram_tensor("wp", [256, 256], bf16, kind="ExternalInput")
    bp = nc.dram_tensor("bp", [1, 512], bf16, kind="ExternalInput")
    onesr = nc.dram_tensor("onesr", [1, 512], bf16, kind="ExternalInput")
    twod = nc.dram_tensor("twod", [1, 2048], bf16, kind="ExternalInput")
    out = nc.dram_tensor("out", [2048, 256], f32, kind="ExternalOutput")

    with tile.TileContext(nc) as tc:
        with (
            tc.tile_pool(name="const", bufs=1) as cp,
            tc.tile_pool(name="actt", bufs=3) as actt,
            tc.tile_pool(name="attp", bufs=16) as attp,
            tc.tile_pool(name="pssc", bufs=2, space="PSUM") as pssc,
            tc.tile_pool(name="psav", bufs=2, space="PSUM") as psav,
            tc.tile_pool(name="psqc", bufs=1, space="PSUM") as psqc,
            tc.tile_pool(name="pspj", bufs=1, space="PSUM") as pspj,
        ):
            # ---- small constants first on the scalar queue so the PE/ACT
            # warmup isn't gated on the bulk-input queues ----
            def load_row(t_dram, shape, tag):
                t = cp.tile(shape, bf16, tag=tag, name=tag)
                nc.scalar.dma_start(t[:], t_dram.ap())
                return t

            ones_sb = load_row(onesr, [1, 512], "ones")
            bq_sb = load_row(bq, [1, 512], "bqr")
            bkv_sb = load_row(bkv, [1, 512], "bkvr")
            bkvc_sb = load_row(bkvcr, [1, 128], "bkvcr")
            bp_sb = load_row(bp, [1, 512], "bpr")

            def load_w(t_dram, shape, tag, rows):
                t = cp.tile(shape, bf16, tag=tag, name=tag)
                nc.sync.dma_start(t[:], t_dram.ap()[rows[0]:rows[1], :])
                return t

            wkv_sb = [load_w(wkv, [128, 128], f"wkv{i}", (i * 128, (i + 1) * 128))
                      for i in range(2)]
            wq_sb = [load_w(wq, [128, 256], f"wq{i}", (i * 128, (i + 1) * 128))
                     for i in range(2)]
            wp_sb = [load_w(wp, [128, 256], f"wp{i}", (i * 128, (i + 1) * 128))
                     for i in range(2)]

            # ---- bulk inputs: split loads, k-conv inputs first ----
            def load_split(t_dram, shape, tag, row0, ncol, piece, eng):
                t = cp.tile(shape, bf16, tag=tag, name=tag)
                for j0 in range(0, ncol, piece):
                    eng.dma_start(
                        t[:, j0:j0 + piece],
                        t_dram.ap()[row0:row0 + 128, j0:j0 + piece])
                return t

            yk_sb = [load_split(yk, [128, 1024], f"yk{i}", i * 128, 1024, 1024,
                                nc.gpsimd) for i in range(2)]
            yv_sb = [load_split(yv, [128, 1024], f"yv{i}", i * 128, 1024, 1024,
                                nc.gpsimd) for i in range(2)]
            xq_sb = [load_split(xq, [128, 2048], f"xq{i}", i * 128, 2048, 1024,
                                nc.scalar if i else nc.sync) for i in range(2)]

            vext = cp.tile([128, 8, 2, 128], bf16, tag="vext")
            nc.gpsimd.dma_start(vext[:], twod.ap().partition_broadcast(128))

            # ---- prime the ACT exp/tanh table set during the input-DMA
            # window (table load ~2.7us; tanh+exp share one set) ----
            wsb = actt.tile([1, 512], f32, tag="warm_sb", name="warm_sb")
            nc.scalar.activation(wsb[:], ones_sb[:], AF.Tanh, scale=0.5)
            nc.scalar.activation(wsb[:], ones_sb[:], AF.Exp, scale=0.03125)

            # ---- conv epilogue: psum holds w@x + bias (preloaded).
            # t = tanh(ps/2) on ACT; dst = (t+1)*ps = 2*silu(ps) on DVE. ----
            def silu_into(ps, dst_ap, ps_ap, tag):
                shape = list(ps.shape)
                t = actt.tile(shape, f32, tag="silu_t", name=f"t_{tag}")
                nc.scalar.activation(t[:], ps, AF.Tanh, scale=0.5)
                nc.vector.scalar_tensor_tensor(
                    out=dst_ap, in0=t[:] if ps_ap is None else ps_ap(t[:]),
                    scalar=1.0, in1=ps if ps_ap is None else ps_ap(ps),
                    op0=ALU.add, op1=ALU.mult)

            # ---- kv conv (k part): kT[pp, m0, m1], pp = hl*64+d ----
            kT = cp.tile([128, 8, 128], bf16, tag="kT")

            def k_conv(mt):
                psw = pssc.tile([128, 1024], f32, tag="sc", name=f"psk{mt}")
                ps = psw[:, 0:512]
                nc.tensor.matmul(ps, lhsT=ones_sb[0:1, 0:128], rhs=bkv_sb[:],
                                 start=True, stop=False, skip_group_check=True)
                for mi in range(4):
                    m0 = 4 * mt + mi
                    sl = slice(mi * 128, (mi + 1) * 128)
                    for c0 in range(2):
                        nc.tensor.matmul(
                            ps[:, sl],
                            lhsT=yk_sb[c0][:, m0 * 128:(m0 + 1) * 128],
                            rhs=wkv_sb[c0][:],
                            start=False, stop=(c0 == 1),
                            skip_group_check=True)
                silu_into(
                    ps, kT[:, 4 * mt:4 * mt + 4, :].rearrange("p a b -> p (a b)"),
                    None, f"k{mt}")

            # ---- kv conv (v part): vext[m1, m0, hl, 0:64] = 2v ----
            def v_conv(jv):
                psw = pspj.tile([128, 512], f32, tag="pj", name=f"psv{jv}")
                ps = psw[:, 0:512]
                nc.tensor.matmul(ps, lhsT=bkvc_sb[:], rhs=ones_sb[:],
                                 start=True, stop=False)
                for c0 in range(2):
                    nc.tensor.matmul(
                        ps, lhsT=wkv_sb[c0][:],
                        rhs=yv_sb[c0][:, jv * 512:(jv + 1) * 512],
                        start=False, stop=(c0 == 1))
                silu_into(
                    ps, vext[:, jv * 4:(jv + 1) * 4, :, 0:64],
                    lambda a: a.rearrange("p (a h d) -> p a h d", a=4, h=2),
                    f"v{jv}")

            # ---- q conv: qT[pp, n0, n1]; t2=0 runs up front (pssc ring),
            # the rest interleave into the attention loop (psqc ring) so the
            # exp stream starts ~15us earlier ----
            qT = cp.tile([128, 16, 256], bf16, tag="qT")

            def q_conv(t2, pool, tag):
                psw = pool.tile([128, 512] if pool is psqc else [128, 1024],
                                f32, tag=tag, name=f"psq{t2}")
                ps = psw[:, 0:512]
                nc.tensor.matmul(ps, lhsT=ones_sb[0:1, 0:128], rhs=bq_sb[:],
                                 start=True, stop=False, skip_group_check=True)
                for nn in range(2):
                    n0 = 2 * t2 + nn
                    sl = slice(nn * 256, (nn + 1) * 256)
                    for c0 in range(2):
                        nc.tensor.matmul(
                            ps[:, sl],
                            lhsT=xq_sb[c0][:, n0 * 128:(n0 + 1) * 128],
                            rhs=wq_sb[c0][:],
                            start=False, stop=(c0 == 1),
                            skip_group_check=True)
                silu_into(
                    ps, qT[:, 2 * t2:2 * t2 + 2, :].rearrange("p a b -> p (a b)"),
                    None, f"q{t2}")

            # ---- attention state ----
            outun = [
                [cp.tile([128, 1024], bf16, tag=f"outun{hl}_{i}",
                         name=f"outun{hl}_{i}") for i in range(2)]
                for hl in range(2)
            ]
            rc_row = [cp.tile([1, 4096], f32, tag=f"rcrow{hl}", name=f"rcrow{hl}")
                      for hl in range(2)]

            att_tiles = {}
            grid_t = {}

            def emit_scores(t2, js=(0, 1, 2, 3)):
                """scores + exp for one n0-pair; hl=0/1 on row groups 0-1/2-3."""
                for j in js:
                    scps = {}
                    for hl in range(2):
                        scps[hl] = pssc.tile([128, 1024], f32, tag="sc",
                                             name=f"scp{hl}_{t2}_{j}")
                    for mi in range(2):
                        m0 = 2 * j + mi
                        for hl in range(2):
                            r0, r1 = hl * 64, (hl + 1) * 64
                            nc.tensor.matmul(
                                scps[hl][:, mi * 512:(mi + 1) * 512],
                                lhsT=kT[r0:r1, m0, :],
                                rhs=qT[r0:r1, 2 * t2:2 * t2 + 2, :],
                                start=True, stop=True)
                    for hl in range(2):
                        a = attp.tile([128, 1024], bf16, tag="att",
                                      name=f"att{hl}_{t2}_{j}")
                        # scoresT = 4*q.k ; want exp(q.k/8) -> scale 1/32
                        nc.scalar.activation(
                            a[:], scps[hl][:], AF.Exp, scale=0.03125)
                        att_tiles[(t2, j, hl)] = a

            def emit_av_pair(t2, j):
                """av matmuls for m0 = 2j, 2j+1, both heads."""
                for hl in range(2):
                    if j == 0:
                        ops = psav.tile([128, 512], f32, tag="ops",
                                        name=f"ops{hl}_{t2}")
                        att_tiles[("ops", t2, hl)] = ops
                    ops = att_tiles[("ops", t2, hl)]
                    for mi in range(2):
                        m0 = 2 * j + mi
                        nc.tensor.matmul(
                            ops[:], lhsT=vext[:, m0, hl, :],
                            rhs=att_tiles[(t2, m0 // 2, hl)][
                                :, (m0 % 2) * 512:(m0 % 2 + 1) * 512],
                            start=(m0 == 0), stop=(m0 == 7))

            def emit_evac(t2):
                """o -> outun (cast), denominators -> rc via fast reciprocal."""
                c0, q0 = t2 & 1, t2 >> 1
                for hl in range(2):
                    ops = att_tiles[("ops", t2, hl)]
                    sl = slice(t2 * 512, (t2 + 1) * 512)
                    sums = actt.tile([1, 512], f32, tag="sums",
                                     name=f"sums{hl}_{t2}")
                    nc.vector.tensor_copy(sums[:], ops[64:65, :])
                    nc.vector.reciprocal_approx_fast(
                        out=rc_row[hl][0:1, sl], in_=sums[:])
                    g = actt.tile([128, 512], f32, tag=f"grid{hl}",
                                  name=f"gr{hl}_{t2}")
                    nc.gpsimd.partition_broadcast(g[:], rc_row[hl][0:1, sl])
                    grid_t[(t2, hl)] = g
                    for nn in range(2):
                        dst = outun[hl][c0][
                            nn * 64:nn * 64 + 64, q0 * 256:(q0 + 1) * 256]
                        nc.vector.tensor_copy(
                            dst, ops[0:64, nn * 256:(nn + 1) * 256])

            def emit_tail(q0):
                """normalize outun columns of quarter q0 and project."""
                for hl in range(2):
                    sl = slice(q0 * 256, (q0 + 1) * 256)
                    for c0 in range(2):
                        g = grid_t.pop((2 * q0 + c0, hl))
                        for band in range(2):
                            pr = slice(band * 64, (band + 1) * 64)
                            nc.vector.tensor_mul(
                                outun[hl][c0][pr, sl], outun[hl][c0][pr, sl],
                                g[pr, band * 256:(band + 1) * 256])
                    pjpool = psqc if (q0 == 3 and hl == 1) else pspj
                    ps2 = pjpool.tile([128, 512], f32,
                                      tag="qc" if (q0 == 3 and hl == 1) else "pj",
                                      name=f"psproj{hl}_{q0}")
                    nc.tensor.matmul(ps2[:], lhsT=ones_sb[0:1, 0:128],
                                     rhs=bp_sb[:], start=True, stop=False,
                                     skip_group_check=True)
                    for half in range(2):
                        fc = 2 * q0 + half
                        sl = slice(half * 256, (half + 1) * 256)
                        for c0 in range(2):
                            nc.tensor.matmul(
                                ps2[:, sl],
                                lhsT=outun[hl][c0][:, fc * 128:(fc + 1) * 128],
                                rhs=wp_sb[c0][:],
                                start=False, stop=(c0 == 1),
                                skip_group_check=True)
                    osb = actt.tile([128, 512], f32, tag="osb",
                                    name=f"osb{hl}_{q0}")
                    nc.vector.tensor_copy(osb[:], ps2[:])
                    dstap = bass.AP(
                        tensor=out,
                        offset=(hl * 1024 + q0) * 256,
                        ap=[[4 * 256, 128], [512 * 256, 2], [1, 256]])
                    nc.sync.dma_start(
                        dstap, osb[:].rearrange("p (h c) -> p h c", h=2))

            # ---- software-pipelined attention: av(t2) first (runnable
            # early in the window), then scores(t2+1) (ring-gated to late
            # window) so the in-order PE queue never head-of-line blocks ----
            k_conv(0)
            k_conv(1)
            v_conv(0)
            v_conv(1)
            q_conv(0, pssc, "sc")
            q_conv(1, psqc, "qc")
            emit_scores(0)
            for t2 in range(8):
                for j in range(4):
                    emit_av_pair(t2, j)
                if t2 < 6:
                    q_conv(t2 + 2, psqc, "qc")
                if t2 < 7:
                    emit_scores(t2 + 1)
                emit_evac(t2)
                if t2 & 1:
                    emit_tail(t2 >> 1)

    nc.compile()
    return nc


def _prep_inputs(x, y, q_w, q_gamma, q_beta, q_mean, q_var,
                 kv_w, kv_gamma, kv_beta, kv_mean, kv_var, proj_w, proj_b):
    f = np.float32
    bf = ml_dtypes.bfloat16
    x = np.ascontiguousarray(np.asarray(x, f))
    y = np.ascontiguousarray(np.asarray(y, f))

    gq = np.asarray(q_gamma, f) / np.sqrt(np.asarray(q_var, f) + BN_EPS)
    bq_full = np.asarray(q_beta, f) - np.asarray(q_mean, f) * gq
    wq_host = np.ascontiguousarray((np.asarray(q_w, f) * gq[:, None]).T).astype(bf)

    gkv = np.asarray(kv_gamma, f) / np.sqrt(np.asarray(kv_var, f) + BN_EPS)
    bkv_full = np.asarray(kv_beta, f) - np.asarray(kv_mean, f) * gkv
    wkv_host = np.ascontiguousarray(
        (np.asarray(kv_w, f) * gkv[:, None]).T).astype(bf)

    wp_host = np.ascontiguousarray(np.asarray(proj_w, f).T).astype(bf)
    bp_host = np.asarray(proj_b, f)

    bq2 = np.tile(bq_full[None, :], (1, 2)).astype(bf)
    bkv2 = np.tile(bkv_full[None, :], (1, 4)).astype(bf)
    bkvc2 = bkv_full[None, :].astype(bf)
    bp2 = np.tile(bp_host[None, :], (1, 2)).astype(bf)
    ones2 = np.ones((1, 512), bf)
    twod2 = np.zeros((1, 2048), np.float32)
    twod2.reshape(16, 128)[:, 64] = 2.0
    twod2 = twod2.astype(bf)

    in_maps = []
    for core in range(8):
        b, hp = core // 2, core % 2
        X4 = x[b].reshape(C, N_TOK)
        Y4 = y[b].reshape(C, N_TOK)
        xqa = np.ascontiguousarray(
            X4.reshape(C, 16, 256)[:, :, hp * 128:(hp + 1) * 128]
        ).reshape(C, 2048).astype(bf)
        Y8 = Y4.reshape(C, 8, 512)
        yka = np.ascontiguousarray(
            Y8[:, :, hp * 128:(hp + 1) * 128]).reshape(C, 1024).astype(bf)
        yva = np.ascontiguousarray(
            Y8[:, :, 256 + hp * 128:256 + (hp + 1) * 128]
        ).reshape(C, 1024).astype(bf)
        in_maps.append({
            "xq": xqa, "yk": yka, "yv": yva,
            "wq": wq_host, "bq": bq2,
            "wkv": wkv_host, "bkv": bkv2, "bkvcr": bkvc2,
            "wp": wp_host, "bp": bp2,
            "onesr": ones2, "twod": twod2,
        })
    return in_maps


def _get_nc():
    if "nc" not in _CACHE:
        _CACHE["nc"] = _build()
    return _CACHE["nc"]


def kernel(x, y, H=64, W=64, q_w=None, q_gamma=None, q_beta=None, q_mean=None,
           q_var=None, kv_w=None, kv_gamma=None, kv_beta=None, kv_mean=None,
           kv_var=None, proj_w=None, proj_b=None, _trace=False):
    from concourse.bass_utils import run_bass_kernel_spmd

    nc = _get_nc()
    in_maps = _prep_inputs(x, y, q_w, q_gamma, q_beta, q_mean, q_var,
                           kv_w, kv_gamma, kv_beta, kv_mean, kv_var,
                           proj_w, proj_b)
    kw = {}
    if _trace:
        kw = {"trace": True, "trace_cores": list(range(8))}
    res = run_bass_kernel_spmd(nc, in_maps, list(range(8)), **kw)
    outa = np.empty((B, N_TOK, C), np.float32)
    for core in range(8):
        b, hp = core // 2, core % 2
        outa[b, hp * 2048:(hp + 1) * 2048, :] = res.results[core]["out"]
    if _trace:
        return outa, res
    return outa
